# revision 40
# baseline (speedup 1.0000x reference)
"""Trainium2 Bass kernel for the LoRA-BC block (nn_LoRABCBlock), v5.4.

Computation (per reference):
    base = x @ w_base.T
    h = layernorm(x) * gamma + beta
    qkv = h @ w_qkv.T ; attention (2 heads, head_dim 32) over full sequence
    attn_out = attn_output @ w_attn_out.T
    delta = ((h + attn_out) @ lora_down) @ lora_up
    out = base + (1/8) * delta

Sharding: data-parallel over (batch, seq-half) -> 8 cores; each core
owns 1024 rows. k/v for the partner's half arrive via a pair-wise
AllGather (replica groups [0,1][2,3][4,5][6,7]) instead of being
recomputed, halving the layernorm / transpose / qkv work per core.

Design notes (vs v4 baseline at ~321-364 us):
  - All weights are packed on the HOST into their final SBUF layouts
    (transposed, bf16, gamma folded into w_qkv and lora_down, beta
    folded into qkv biases and a constant LoRA row). Weight DMAs ride
    the gpsimd software DGE so their descriptor generation never
    blocks the Scalar/Sync sequencers.
  - zhT holds z' = (x * rstd)^T, produced by PE transposes with
    rhs = diag(rstd). Mean subtraction is exact algebra downstream:
    base = z'@WbT * std needs no correction; qkv / lora-down get a
    rank-1 colsum x mr-row correction matmul in their psum group.
  - attn_out only reaches the output through the rank-8 LoRA, so
    w_attn_out @ lora_down collapses to a 64x8 matrix (LW2).
  - Final output = baseS + aug_tT.T @ aug_lu with aug_tT = [8 lora
    rows; ...; ones] and aug_lu = [SCALING*lu; ...; beta const].
  - x uploaded bf16; LN stats via bn_stats on bf16.
  - PE kept continuously busy (p-state): base matmul tiles fill the
    LN phase, the gather window, and attention rounds.
"""

import sys

sys.path.insert(0, "/opt/trn_rl_repo")

from contextlib import ExitStack

import ml_dtypes
import numpy as np

import concourse.bass as bass
import concourse.tile as tile
from concourse import bacc, mybir
from concourse.bass_utils import run_bass_kernel_spmd
from concourse.masks import make_identity

F32 = mybir.dt.float32
BF16 = mybir.dt.bfloat16
AF = mybir.ActivationFunctionType
ALU = mybir.AluOpType
BF16NP = ml_dtypes.bfloat16

E = 1024          # embed dim
DM = 1024         # d_model
R = 8             # lora rank
SCALING = 1.0 / R
DA = 64           # attn dim
NH = 2            # heads
HD = DA // NH     # head dim = 32
SOWN = 1024       # rows owned per core
SFULL = 2048      # rows per batch element
NC = 8            # cores
P = 128
KT = E // P       # 8 k-tiles
MT = SOWN // P    # 8 own m-tiles
ST = SFULL // P   # 16 sequence tiles (full, post-gather)
MTO = SOWN // P   # 8 own sequence tiles
ATT_SCALE = float(HD) ** -0.5
AUG = 33          # rows 0..7 lora tT, 8..31 zero, 32 ones


def build_kernel(dbg=False):
    nc = bacc.Bacc("TRN2", target_bir_lowering=False, debug=False, num_devices=NC)

    x_own = nc.dram_tensor("x_own", [SOWN, E], BF16, kind="ExternalInput").ap()
    wqkvT_d = nc.dram_tensor("wqkvT", [P, KT, 3 * DA], BF16, kind="ExternalInput").ap()
    qkvb_d = nc.dram_tensor("qkvb", [P, 3], F32, kind="ExternalInput").ap()
    csum_d = nc.dram_tensor("csum", [1, 256], BF16, kind="ExternalInput").ap()
    WbT_d = nc.dram_tensor("WbT", [P, KT, DM], BF16, kind="ExternalInput").ap()
    ldg_d = nc.dram_tensor("ldg", [P, KT, R], BF16, kind="ExternalInput").ap()
    LW2_d = nc.dram_tensor("LW2", [DA, R], BF16, kind="ExternalInput").ap()
    auglu_d = nc.dram_tensor("auglu", [AUG, DM], BF16, kind="ExternalInput").ap()
    out_d = nc.dram_tensor("out", [SOWN, DM], BF16, kind="ExternalOutput").ap()
    dbg_d = {}
    if dbg:
        for nm, shp in [("zhT", [P, KT, SOWN]), ("qT", [DA, SOWN]),
                        ("kvT", [P, SFULL]),
                        ("vaug0", [P, ST, 48]), ("vaug1", [P, ST, 48]),
                        ("aoT", [DA, SOWN]), ("augT", [AUG, SOWN]),
                        ("baseS", [P, MT, DM]), ("mrRow", [1, SOWN])]:
            dbg_d[nm] = nc.dram_tensor("dbg_" + nm, shp, BF16,
                                       kind="ExternalOutput").ap()

    with tile.TileContext(nc) as tc, ExitStack() as ctx:
        persist = ctx.enter_context(tc.tile_pool(name="persist", bufs=1))
        ld_pool = ctx.enter_context(tc.tile_pool(name="loads", bufs=3))
        st_pool = ctx.enter_context(tc.tile_pool(name="stats", bufs=4))
        dg_pool = ctx.enter_context(tc.tile_pool(name="diags", bufs=3))
        ex_pool = ctx.enter_context(tc.tile_pool(name="expb", bufs=2))
        o_pool = ctx.enter_context(tc.tile_pool(name="outs", bufs=4))
        dram = ctx.enter_context(tc.tile_pool(name="dram", bufs=1, space="DRAM"))
        ps = ctx.enter_context(tc.tile_pool(name="ps", bufs=1, space="PSUM"))

        _psn = [0]
        ps_ph1 = [None]
        ps_mid = [None]
        ps_attn = [None]
        ps_tail = [None]

        def pstile(tag, shape, bufs, dtype=F32, pool=None):
            _psn[0] += 1
            return (pool or ps).tile(shape, dtype, tag=tag, bufs=bufs,
                                     name=f"ps_{tag}_{_psn[0]}")

        def ps_psc():      # 2 banks x 2 bufs: attn scores (attn pool)
            return pstile("psc", [P, 2, 512], 2, pool=ps_attn[0])

        def ps_acc():      # 1 bank x 2 bufs: qkv + base pot (ph1 pool)
            return pstile("acc", [P, 512], 2, pool=ps_ph1[0])

        def ps_accb():     # 1 bank: attention-phase pots + rest (attn pool)
            return pstile("accb", [P, 512], 1, pool=ps_attn[0])

        def ps_sm():       # 1 bank: mr rows + lora tT psum
            return pstile("sm", [HD + 1, 512], 1)

        def ps_pao():      # 1 bank: attn @ v accumulator
            return pstile("pao", [HD + 1, 512], 1)

        def ps_tpz():      # 2 banks x 2 bufs: z transposes (ph1 pool)
            return pstile("tpz", [P, 8, P], 2, pool=ps_ph1[0])

        def ps_tpv():      # 1 bank, bf16: v transposes (attn pool)
            return pstile("tpv", [P, 512], 1, dtype=BF16, pool=ps_attn[0])

        # ---------------- constants ----------------
        ident = persist.tile([P, P], BF16, tag="ident")
        make_identity(nc, ident)
        eps_t = persist.tile([P, 1], F32, tag="eps")
        nc.vector.memset(eps_t, 1e-5)
        # prewarm every activation-table set used, before anything queues
        NQ = 2                     # gather chunks
        QW = SOWN // NQ            # columns per chunk
        kv_in = [dram.tile([P, QW], BF16, name=f"kv_in{i}") for i in range(NQ)]
        kv_out = [dram.tile([2, P, QW], BF16, name=f"kv_out{i}")
                  for i in range(NQ)]
        warm = persist.tile([1, 4], F32, tag="warm")
        nc.scalar.activation(out=warm[:, 1:2], in_=eps_t[0:1, :], func=AF.Sqrt,
                             bias=eps_t[0:1, :])
        nc.scalar.activation(out=warm[:, 2:3], in_=eps_t[0:1, :],
                             func=AF.Identity, bias=eps_t[0:1, :])
        nc.scalar.activation(out=warm[:, 3:4], in_=eps_t[0:1, :], func=AF.Copy)
        nc.scalar.activation(out=warm[:, 0:1], in_=eps_t[0:1, :], func=AF.Exp)

        # ---------------- weights (host-packed; gpsimd DGE DMAs) --------
        wqkvT = persist.tile([P, KT, 3 * DA], BF16, tag="wqkvT")
        nc.gpsimd.dma_start(out=wqkvT, in_=wqkvT_d)
        qkvb = persist.tile([P, 3], F32, tag="qkvb")
        nc.gpsimd.dma_start(out=qkvb, in_=qkvb_d)
        csum = persist.tile([1, 256], BF16, tag="csum")
        nc.gpsimd.dma_start(out=csum, in_=csum_d)
        ldg_sb = persist.tile([P, KT, R], BF16, tag="ldg")
        nc.gpsimd.dma_start(out=ldg_sb, in_=ldg_d)
        LW2_sb = persist.tile([DA, R], BF16, tag="LW2")
        nc.gpsimd.dma_start(out=LW2_sb, in_=LW2_d)
        auglu_sb = persist.tile([AUG, DM], BF16, tag="auglu")
        nc.gpsimd.dma_start(out=auglu_sb, in_=auglu_d)
        WbT = persist.tile([P, KT, DM], BF16, tag="WbT")
        for k in range(KT):
            nc.gpsimd.dma_start(out=WbT[:, k, :], in_=WbT_d[:, k, :])

        # ---------------- persistent activations ----------------
        zhT = persist.tile([P, KT, SOWN], BF16, tag="zhT")    # z' = x*rstd, T
        qT = persist.tile([DA, SOWN], BF16, tag="qT")
        kv_own = persist.tile([P, SOWN], BF16, tag="kv_own")  # k 0:64, v 64:128
        kvT = persist.tile([P, SFULL], BF16, tag="kvT")       # global order
        kTt = kvT[0:DA, :]
        vT = kvT[DA:P, :]
        v_aug = [persist.tile([P, ST, 48], BF16, tag=f"vaug{h}", name=f"vaug{h}")
                 for h in range(NH)]
        aoT = persist.tile([DA, SOWN], BF16, tag="aoT")
        aug_tT = persist.tile([AUG, SOWN], BF16, tag="aug_tT")
        baseS = persist.tile([P, MT, DM], BF16, tag="baseS")
        stdAll = persist.tile([P, MT], F32, tag="stdAll")
        mrRow = persist.tile([1, SOWN], BF16, tag="mrRow")    # +mu*rstd
        rrS = persist.tile([HD, 512], F32, tag="rrS")
        nc.vector.memset(rrS, 0.0)

        for h in range(NH):
            nc.gpsimd.memset(v_aug[h][:, :, HD:HD + 1], 1.0)
        nc.gpsimd.memset(aug_tT, 0.0)
        nc.gpsimd.memset(aug_tT[32:33, :], 1.0)

        # ---------------- phase 1: x load + layernorm + z'^T ------------
        xin = persist.tile([P, MTO, E], BF16, tag="xin")
        for st in range(MTO):
            nc.sync.dma_start(out=xin[:, st, :],
                              in_=x_own[st * P:(st + 1) * P, :])

        def do_st(st):
            xf = xin[:, st, :]
            stats = st_pool.tile([P, 2, 6], F32, tag="bnstats")
            xr = xf.rearrange("p (n f) -> p n f", f=512)
            for sg in range(2):
                nc.vector.bn_stats(out=stats[:, sg, :], in_=xr[:, sg, :])
            mv = st_pool.tile([P, 2], F32, tag="mv")
            nc.vector.bn_aggr(out=mv, in_=stats)
            nc.scalar.activation(out=stdAll[:, st:st + 1], in_=mv[:, 1:2],
                                 func=AF.Sqrt, bias=eps_t)
            rstd = st_pool.tile([P, 1], F32, tag="rstd")
            nc.vector.reciprocal(out=rstd, in_=stdAll[:, st:st + 1])
            mr = st_pool.tile([P, 1], BF16, tag="mr")
            nc.scalar.activation(out=mr, in_=mv[:, 0:1], func=AF.Identity,
                                 scale=rstd)
            diag = dg_pool.tile([P, P], BF16, tag="diag")
            nc.vector.tensor_scalar(out=diag, in0=ident, scalar1=rstd,
                                    scalar2=None, op0=ALU.mult)
            smt = ps_sm()
            nc.tensor.matmul(smt[0:1, 0:P], mr, ident, start=True, stop=True)
            if st % 2 == 0:
                nc.vector.tensor_copy(out=mrRow[0:1, st * P:(st + 1) * P],
                                      in_=smt[0:1, 0:P])
            else:
                nc.scalar.copy(out=mrRow[0:1, st * P:(st + 1) * P],
                               in_=smt[0:1, 0:P])
            tpz = ps_tpz()
            tpf = tpz.rearrange("p j f -> p (j f)")
            for k in range(KT):
                nc.tensor.matmul(tpf[:, k * P:(k + 1) * P],
                                 xf[:, k * P:(k + 1) * P], diag,
                                 start=True, stop=True)
            tpk = tpz.rearrange("p j a -> p (j a)").rearrange(
                "p (k a) -> p k a", a=P)
            if st % 2 == 0:
                nc.vector.tensor_copy(out=zhT[:, :, st * P:(st + 1) * P],
                                      in_=tpk)
            else:
                nc.scalar.copy(out=zhT[:, :, st * P:(st + 1) * P], in_=tpk)

        # ---------------- qkv (own half only) ----------------
        def qk_own(grp):
            pq = ps_acc()
            for k in range(KT):
                nc.tensor.matmul(pq, wqkvT[:, k, 0:P],
                                 zhT[:, k, grp * 512:(grp + 1) * 512],
                                 start=(k == 0), stop=False)
            nc.tensor.matmul(pq, csum[0:1, 0:P],
                             mrRow[0:1, grp * 512:(grp + 1) * 512],
                             start=False, stop=True)
            nc.vector.tensor_scalar(
                out=qT[:, grp * 512:(grp + 1) * 512], in0=pq[0:DA, :],
                scalar1=qkvb[0:DA, 0:1], scalar2=None, op0=ALU.add)
            nc.scalar.activation(
                out=kv_own[0:DA, grp * 512:(grp + 1) * 512], in_=pq[DA:P, :],
                func=AF.Identity, bias=qkvb[DA:P, 0:1])

        def qkv_v(grp):
            pv = ps_acc()
            for k in range(KT):
                nc.tensor.matmul(pv[0:DA, :], wqkvT[:, k, P:3 * DA],
                                 zhT[:, k, grp * 512:(grp + 1) * 512],
                                 start=(k == 0), stop=False)
            nc.tensor.matmul(pv[0:DA, :], csum[0:1, P:3 * DA],
                             mrRow[0:1, grp * 512:(grp + 1) * 512],
                             start=False, stop=True)
            nc.vector.tensor_scalar(
                out=kv_own[DA:P, grp * 512:(grp + 1) * 512], in0=pv[0:DA, :],
                scalar1=qkvb[0:DA, 1:2], scalar2=None, op0=ALU.add)

        def vaug_grp(g):  # transpose v tiles jt = 4g..4g+3 (local order)
            tpv = ps_tpv()
            for j in range(4):
                jt = g * 4 + j
                nc.tensor.transpose(tpv[:, j * DA:(j + 1) * DA],
                                    vT[:, jt * P:(jt + 1) * P],
                                    ident[DA:P, DA:P])
            tv = tpv.rearrange("p (j d) -> p j d", d=DA)
            for h in range(NH):
                nc.vector.tensor_copy(
                    out=v_aug[h][:, g * 4:(g + 1) * 4, 0:HD],
                    in_=tv[:, 0:4, h * HD:(h + 1) * HD])

        # ---------------- base matmul tile ----------------
        in_attn = [False]
        pot_ps = [None]

        def pot_half(mt, g, half):
            if half == 0:
                pot_ps[0] = ps_accb() if in_attn[0] else ps_acc()
            pot = pot_ps[0]
            for k in range(half * 4, half * 4 + 4):
                nc.tensor.matmul(pot, zhT[:, k, mt * P:(mt + 1) * P],
                                 WbT[:, k, g * 512:(g + 1) * 512],
                                 start=(k == 0), stop=(k == KT - 1))
            if half == 1:
                if g == 0:
                    nc.vector.tensor_scalar(
                        out=baseS[:, mt, g * 512:(g + 1) * 512], in0=pot,
                        scalar1=stdAll[:, mt:mt + 1], scalar2=None,
                        op0=ALU.mult)
                else:
                    nc.scalar.activation(
                        out=baseS[:, mt, g * 512:(g + 1) * 512], in_=pot,
                        func=AF.Copy, scale=stdAll[:, mt:mt + 1])

        def pot_mt(mt, g):
            pot_half(mt, g, 0)
            pot_half(mt, g, 1)

        # ---------------- k/v pair exchange (quartered AllGather) -------
        # only the first collective pays the ~11us dispatch latency; the
        # rest queue behind it and pipeline on the pair link
        def kv_exchange(qc):
            c0 = qc * QW
            nc.sync.dma_start(out=kv_in[qc], in_=kv_own[:, c0:c0 + QW])
            nc.gpsimd.collective_compute(
                "AllGather", ALU.bypass,
                replica_groups=[[2 * i, 2 * i + 1] for i in range(NC // 2)],
                ins=[kv_in[qc].opt()], outs=[kv_out[qc].opt()])

        def kv_load(qc):
            kvv = kvT.rearrange("p (q g s) -> p q g s", q=NQ, g=2)
            nc.sync.dma_start(out=kvv[:, qc, :, :],
                              in_=kv_out[qc].rearrange("g p s -> p g s"))

        # ---------------- phase 1 schedule ----------------
        ps_ph1[0] = tc.alloc_tile_pool(name="ps_ph1", bufs=1, space="PSUM")
        extra = {3: [lambda: qk_own(0), lambda: qkv_v(0),
                     lambda: kv_exchange(0)],
                 4: [lambda: pot_mt(0, 0), lambda: pot_mt(0, 1)],
                 6: [lambda: pot_mt(1, 0), lambda: pot_mt(1, 1)],
                 7: [lambda: qk_own(1), lambda: qkv_v(1),
                     lambda: kv_exchange(1)]}
        for st in range(MTO):
            do_st(st)
            for fn in extra.get(st, []):
                fn()

        # cover the gather with two more base tiles, then load + transpose
        pot_half(2, 0, 0)
        pot_half(2, 0, 1)
        kv_load(0)
        pot_half(2, 1, 0)
        pot_half(2, 1, 1)
        pot_half(3, 0, 0)
        pot_half(3, 0, 1)
        kv_load(1)
        pot_half(3, 1, 0)
        pot_half(3, 1, 1)
        ps_ph1[0].release()

        in_attn[0] = True
        ps_attn[0] = tc.alloc_tile_pool(name="ps_attn", bufs=1, space="PSUM")
        vaug_grp(0)
        vaug_grp(1)

        # ---------------- attention (+ interleaved base / rest) ---------
        # skt pairs ordered so the first gather chunk's keys (global cols
        # 0:512 and 1024:1536) are consumed first; softmax is order-invariant
        # kvT is chunk-major: local tiles 0..7 are gather chunk 0, 8..15
        # chunk 1 (global positions scrambled; softmax is order-invariant)
        SKT_ORDER = list(range(ST))

        def attn_block(h, qg, fillers, preburst=()):
            d0 = h * HD
            pao = ps_pao()
            for fn in preburst:
                fn()
            exts = []
            nf = len(fillers)
            fi = 0

            def av(r):
                for j in range(2):
                    i = r * 2 + j
                    skt = SKT_ORDER[i]
                    nc.tensor.matmul(pao, v_aug[h][:, skt, 0:HD + 1],
                                     exts[r][:, j, :],
                                     start=(i == 0), stop=(i == ST - 1))

            for r in range(8):
                psc = ps_psc()
                for j in range(2):
                    skt = SKT_ORDER[r * 2 + j]
                    nc.tensor.matmul(psc[:, j, :],
                                     kTt[d0:d0 + HD, skt * P:(skt + 1) * P],
                                     qT[d0:d0 + HD, qg * 512:(qg + 1) * 512],
                                     start=True, stop=True)
                ext = ex_pool.tile([P, 2, 512], BF16, tag="ext")
                nc.scalar.activation(out=ext, in_=psc, func=AF.Exp,
                                     scale=ATT_SCALE)
                exts.append(ext)
                if r > 0:
                    av(r - 1)
                while fi * 8 < nf * (r + 1):
                    fillers[fi]()
                    fi += 1
            av(7)
            nc.vector.reciprocal(out=rrS[0:1, :], in_=pao[HD:HD + 1, :])
            rrb = st_pool.tile([HD, 512], F32, tag="rrb")
            # broadcast lane 0 across the 32-partition group on the DVE:
            # keeps the finalize on one engine (no Pool hop + sem trips)
            nc.vector.stream_shuffle(out=rrb, in_=rrS, mask=[0] * 32)
            nc.vector.tensor_tensor(
                out=aoT[d0:d0 + HD, qg * 512:(qg + 1) * 512],
                in0=pao[0:HD, :], in1=rrb, op=ALU.mult)
            while fi < nf:
                fillers[fi]()
                fi += 1

        tT_ps = [None]

        def tT_ldg(qg):
            p5 = ps_sm()[0:R, :]
            tT_ps[0] = p5
            for k in range(KT):
                nc.tensor.matmul(p5, ldg_sb[:, k, :],
                                 zhT[:, k, qg * 512:(qg + 1) * 512],
                                 start=(k == 0), stop=False)
            nc.tensor.matmul(p5, csum[0:1, 192:200],
                             mrRow[0:1, qg * 512:(qg + 1) * 512],
                             start=False, stop=False)

        def tT_h0(qg):
            nc.tensor.matmul(tT_ps[0], LW2_sb[0:HD, :],
                             aoT[0:HD, qg * 512:(qg + 1) * 512],
                             start=False, stop=False)

        def tT_part2(qg):
            p5 = tT_ps[0]
            nc.tensor.matmul(p5, LW2_sb[HD:DA, :],
                             aoT[HD:DA, qg * 512:(qg + 1) * 512],
                             start=False, stop=True)
            nc.vector.tensor_copy(
                out=aug_tT[0:R, qg * 512:(qg + 1) * 512], in_=p5)

        def rest_g(mt, g):
            racc = ps_accb()
            nc.tensor.matmul(racc, aug_tT[:, mt * P:(mt + 1) * P],
                             auglu_sb[:, g * 512:(g + 1) * 512],
                             start=True, stop=True)
            o_t = o_pool.tile([P, 512], BF16, tag="o_t")
            nc.vector.tensor_tensor(out=o_t, in0=racc,
                                    in1=baseS[:, mt, g * 512:(g + 1) * 512],
                                    op=ALU.add)
            nc.sync.dma_start(
                out=out_d[mt * P:(mt + 1) * P, g * 512:(g + 1) * 512],
                in_=o_t)

        def rest_tail_all(npre=0):
            # software-pipelined tail over (mt, g) items: ACT pre-copy of
            # baseS into psum, aug matmul accumulates on top, DVE copy out.
            # raccs ride the freed psc ring, two halves per alloc.
            items = [(mt, g) for mt in range(4, MT) for g in range(2)]
            raccs = [None] * len(items)
            pts = []
            for i, (mt, g) in enumerate(items):
                if i % 2 == 0:
                    pts.append(ps_psc())
                raccs[i] = pts[-1][:, i % 2, :]

            def pre(i):
                mt, g = items[i]
                nc.scalar.activation(
                    out=raccs[i], in_=baseS[:, mt, g * 512:(g + 1) * 512],
                    func=AF.Copy)

            def mm(i):
                mt, g = items[i]
                nc.tensor.matmul(raccs[i], aug_tT[:, mt * P:(mt + 1) * P],
                                 auglu_sb[:, g * 512:(g + 1) * 512],
                                 start=False, stop=True, skip_group_check=True)

            def out(i):
                mt, g = items[i]
                o_t = o_pool.tile([P, 512], BF16, tag="o_t")
                nc.vector.tensor_copy(out=o_t, in_=raccs[i])
                nc.sync.dma_start(
                    out=out_d[mt * P:(mt + 1) * P, g * 512:(g + 1) * 512],
                    in_=o_t)

            n = len(items)
            for i in range(npre):
                pre(i)
            for i in range(n + 2):
                if npre <= i < n:
                    pre(i)
                if 1 <= i < n + 1:
                    mm(i - 1)
                if i >= 2:
                    out(i - 2)

        attn_block(0, 0, [lambda: vaug_grp(2), lambda: vaug_grp(3),
                          lambda: pot_half(4, 0, 0), lambda: pot_half(4, 0, 1),
                          lambda: pot_half(4, 1, 0), lambda: pot_half(4, 1, 1)])
        attn_block(1, 0, [lambda: pot_half(5, 0, 0), lambda: pot_half(5, 0, 1),
                          lambda: pot_half(5, 1, 0), lambda: pot_half(5, 1, 1),
                          lambda: pot_half(6, 0, 0), lambda: pot_half(6, 0, 1)])
        attn_block(0, 1, [lambda: pot_half(6, 1, 0), lambda: pot_half(6, 1, 1),
                          lambda: tT_ldg(0), lambda: tT_h0(0),
                          lambda: tT_part2(0), lambda: rest_g(0, 0),
                          lambda: rest_g(0, 1), lambda: rest_g(1, 0)])
        attn_block(1, 1, [lambda: tT_ldg(1), lambda: pot_half(7, 0, 0),
                          lambda: pot_half(7, 0, 1), lambda: rest_g(1, 1),
                          lambda: tT_h0(1), lambda: pot_half(7, 1, 0),
                          lambda: pot_half(7, 1, 1), lambda: rest_g(2, 0),
                          lambda: rest_g(2, 1), lambda: rest_g(3, 0),
                          lambda: rest_g(3, 1)])
        tail_fns = rest_tail_all
        # stage the first tail pre-copies while the last finalize drains
        tT_part2(1)
        rest_tail_all(npre=3)
        ps_attn[0].release()

        if dbg:
            for nm, sb in [("zhT", zhT), ("qT", qT), ("kvT", kvT),
                           ("vaug0", v_aug[0]), ("vaug1", v_aug[1]),
                           ("aoT", aoT), ("augT", aug_tT),
                           ("baseS", baseS), ("mrRow", mrRow)]:
                nc.sync.dma_start(out=dbg_d[nm], in_=sb)

    nc.compile()
    return nc


_NC_CACHE = None


def _get_nc():
    global _NC_CACHE
    if _NC_CACHE is None:
        _NC_CACHE = build_kernel()
    return _NC_CACHE


def pack_weights(w_base, ln_gamma, ln_beta, lora_down, lora_up, w_qkv,
                 w_attn_out):
    """Host-side packing of all weights into device SBUF layouts."""
    f8 = np.float64
    g = np.asarray(ln_gamma, f8)
    b = np.asarray(ln_beta, f8)
    Wq = np.asarray(w_qkv, f8)                       # [192, E]
    Wqg = Wq * g[None, :]                            # gamma-folded
    wqkvT = Wqg.T.reshape(KT, P, 3 * DA).transpose(1, 0, 2)
    bqkv = b @ Wq.T                                  # [192]
    qkvb = np.zeros((P, 3), np.float32)
    qkvb[0:DA, 0] = bqkv[0:DA]                       # bq (partitions 0:64)
    qkvb[DA:P, 0] = bqkv[DA:2 * DA]                  # bk (partitions 64:128)
    qkvb[0:DA, 1] = bqkv[2 * DA:3 * DA]              # bv (partitions 0:64)
    qkvb[0:DA, 2] = bqkv[DA:2 * DA]                  # bk (partitions 0:64)
    Wb = np.asarray(w_base, f8)                      # [DM, E]
    WbT = Wb.T.reshape(KT, P, DM).transpose(1, 0, 2)
    ld = np.asarray(lora_down, f8)                   # [E, R]
    ldgm = ld * g[:, None]                           # gamma-folded
    ldg = ldgm.reshape(KT, P, R).transpose(1, 0, 2)
    LW2 = np.asarray(w_attn_out, f8).T @ ld          # [DA, R]
    lu = np.asarray(lora_up, f8)                     # [R, DM]
    csum = np.zeros((1, 256), f8)
    # negated: device row is +mu*rstd, correction subtracts colsum * mr
    csum[0, 0:3 * DA] = -Wqg.sum(axis=1)
    csum[0, 3 * DA:3 * DA + R] = -ldgm.sum(axis=0)
    auglu = np.zeros((AUG, DM), f8)
    auglu[0:R] = SCALING * lu
    auglu[32] = SCALING * ((b @ ld) @ lu)            # beta const (pairs ones)

    def cast(a):
        return np.ascontiguousarray(a.astype(BF16NP))

    return {"wqkvT": cast(wqkvT), "qkvb": np.ascontiguousarray(qkvb),
            "csum": cast(csum), "WbT": cast(WbT), "ldg": cast(ldg),
            "LW2": cast(LW2), "auglu": cast(auglu)}


def core_in_maps(x, wk):
    xb = np.asarray(x).astype(BF16NP)
    in_maps = []
    for c in range(NC):
        b, half = divmod(c, 2)
        own = np.ascontiguousarray(xb[b, half * SOWN:(half + 1) * SOWN])
        in_maps.append({"x_own": own, **wk})
    return in_maps


def kernel(x, w_base, ln_gamma, ln_beta, lora_down, lora_up, w_qkv, w_attn_out,
           _trace=False):
    wk = pack_weights(w_base, ln_gamma, ln_beta, lora_down, lora_up, w_qkv,
                      w_attn_out)
    nc = _get_nc()
    in_maps = core_in_maps(x, wk)
    res = run_bass_kernel_spmd(nc, in_maps, core_ids=list(range(NC)),
                               trace=_trace)
    B, S = 4, SFULL
    out = np.empty((B, S, DM), np.float32)
    for c in range(NC):
        b, half = divmod(c, 2)
        out[b, half * SOWN:(half + 1) * SOWN] = np.asarray(
            res.results[c]["out"], dtype=np.float32)
    if _trace:
        kernel.last_exec_time_ns = res.exec_time_ns
        kernel.last_results = res
    return out


# revision 41
# speedup vs baseline: 1.0149x; 1.0149x over previous
"""Trainium2 Bass kernel for the LoRA-BC block (nn_LoRABCBlock), v5.4.

Computation (per reference):
    base = x @ w_base.T
    h = layernorm(x) * gamma + beta
    qkv = h @ w_qkv.T ; attention (2 heads, head_dim 32) over full sequence
    attn_out = attn_output @ w_attn_out.T
    delta = ((h + attn_out) @ lora_down) @ lora_up
    out = base + (1/8) * delta

Sharding: data-parallel over (batch, seq-half) -> 8 cores; each core
owns 1024 rows. k/v for the partner's half arrive via a pair-wise
AllGather (replica groups [0,1][2,3][4,5][6,7]) instead of being
recomputed, halving the layernorm / transpose / qkv work per core.

Design notes (vs v4 baseline at ~321-364 us):
  - All weights are packed on the HOST into their final SBUF layouts
    (transposed, bf16, gamma folded into w_qkv and lora_down, beta
    folded into qkv biases and a constant LoRA row). Weight DMAs ride
    the gpsimd software DGE so their descriptor generation never
    blocks the Scalar/Sync sequencers.
  - zhT holds z' = (x * rstd)^T, produced by PE transposes with
    rhs = diag(rstd). Mean subtraction is exact algebra downstream:
    base = z'@WbT * std needs no correction; qkv / lora-down get a
    rank-1 colsum x mr-row correction matmul in their psum group.
  - attn_out only reaches the output through the rank-8 LoRA, so
    w_attn_out @ lora_down collapses to a 64x8 matrix (LW2).
  - Final output = baseS + aug_tT.T @ aug_lu with aug_tT = [8 lora
    rows; ...; ones] and aug_lu = [SCALING*lu; ...; beta const].
  - x uploaded bf16; LN stats via bn_stats on bf16.
  - PE kept continuously busy (p-state): base matmul tiles fill the
    LN phase, the gather window, and attention rounds.
"""

import sys

sys.path.insert(0, "/opt/trn_rl_repo")

from contextlib import ExitStack

import ml_dtypes
import numpy as np

import concourse.bass as bass
import concourse.tile as tile
from concourse import bacc, mybir
from concourse.bass_utils import run_bass_kernel_spmd
from concourse.masks import make_identity

F32 = mybir.dt.float32
BF16 = mybir.dt.bfloat16
AF = mybir.ActivationFunctionType
ALU = mybir.AluOpType
BF16NP = ml_dtypes.bfloat16

E = 1024          # embed dim
DM = 1024         # d_model
R = 8             # lora rank
SCALING = 1.0 / R
DA = 64           # attn dim
NH = 2            # heads
HD = DA // NH     # head dim = 32
SOWN = 1024       # rows owned per core
SFULL = 2048      # rows per batch element
NC = 8            # cores
P = 128
KT = E // P       # 8 k-tiles
MT = SOWN // P    # 8 own m-tiles
ST = SFULL // P   # 16 sequence tiles (full, post-gather)
MTO = SOWN // P   # 8 own sequence tiles
ATT_SCALE = float(HD) ** -0.5
AUG = 33          # rows 0..7 lora tT, 8..31 zero, 32 ones


def build_kernel(dbg=False):
    nc = bacc.Bacc("TRN2", target_bir_lowering=False, debug=False, num_devices=NC)

    x_own = nc.dram_tensor("x_own", [SOWN, E], BF16, kind="ExternalInput").ap()
    wqkvT_d = nc.dram_tensor("wqkvT", [P, KT, 3 * DA], BF16, kind="ExternalInput").ap()
    qkvb_d = nc.dram_tensor("qkvb", [P, 3], F32, kind="ExternalInput").ap()
    csum_d = nc.dram_tensor("csum", [1, 256], BF16, kind="ExternalInput").ap()
    WbT_d = nc.dram_tensor("WbT", [P, KT, DM], BF16, kind="ExternalInput").ap()
    ldg_d = nc.dram_tensor("ldg", [P, KT, R], BF16, kind="ExternalInput").ap()
    LW2_d = nc.dram_tensor("LW2", [DA, R], BF16, kind="ExternalInput").ap()
    auglu_d = nc.dram_tensor("auglu", [AUG, DM], BF16, kind="ExternalInput").ap()
    out_d = nc.dram_tensor("out", [SOWN, DM], BF16, kind="ExternalOutput").ap()
    dbg_d = {}
    if dbg:
        for nm, shp in [("zhT", [P, KT, SOWN]), ("qT", [DA, SOWN]),
                        ("kvT", [P, SFULL]),
                        ("vaug0", [P, ST, 48]), ("vaug1", [P, ST, 48]),
                        ("aoT", [DA, SOWN]), ("augT", [AUG, SOWN]),
                        ("baseS", [P, MT, DM]), ("mrRow", [1, SOWN])]:
            dbg_d[nm] = nc.dram_tensor("dbg_" + nm, shp, BF16,
                                       kind="ExternalOutput").ap()

    with tile.TileContext(nc) as tc, ExitStack() as ctx:
        persist = ctx.enter_context(tc.tile_pool(name="persist", bufs=1))
        ld_pool = ctx.enter_context(tc.tile_pool(name="loads", bufs=3))
        st_pool = ctx.enter_context(tc.tile_pool(name="stats", bufs=4))
        dg_pool = ctx.enter_context(tc.tile_pool(name="diags", bufs=3))
        ex_pool = ctx.enter_context(tc.tile_pool(name="expb", bufs=2))
        o_pool = ctx.enter_context(tc.tile_pool(name="outs", bufs=4))
        dram = ctx.enter_context(tc.tile_pool(name="dram", bufs=1, space="DRAM"))
        ps = ctx.enter_context(tc.tile_pool(name="ps", bufs=1, space="PSUM"))

        _psn = [0]
        ps_ph1 = [None]
        ps_mid = [None]
        ps_attn = [None]
        ps_tail = [None]

        def pstile(tag, shape, bufs, dtype=F32, pool=None):
            _psn[0] += 1
            return (pool or ps).tile(shape, dtype, tag=tag, bufs=bufs,
                                     name=f"ps_{tag}_{_psn[0]}")

        def ps_psc():      # 2 banks x 2 bufs: attn scores (attn pool)
            return pstile("psc", [P, 2, 512], 2, pool=ps_attn[0])

        def ps_acc():      # 1 bank x 2 bufs: qkv + base pot (ph1 pool)
            return pstile("acc", [P, 512], 2, pool=ps_ph1[0])

        def ps_accb():     # 1 bank: attention-phase pots + rest (attn pool)
            return pstile("accb", [P, 512], 1, pool=ps_attn[0])

        def ps_sm():       # 1 bank: mr rows + lora tT psum
            return pstile("sm", [HD + 1, 512], 1)

        def ps_pao():      # 1 bank: attn @ v accumulator
            return pstile("pao", [HD + 1, 512], 1)

        def ps_tpz():      # 2 banks x 2 bufs: z transposes (ph1 pool)
            return pstile("tpz", [P, 8, P], 2, pool=ps_ph1[0])

        def ps_tpv():      # 1 bank, bf16: v transposes (attn pool)
            return pstile("tpv", [P, 512], 1, dtype=BF16, pool=ps_attn[0])

        # ---------------- constants ----------------
        ident = persist.tile([P, P], BF16, tag="ident")
        make_identity(nc, ident)
        eps_t = persist.tile([P, 1], F32, tag="eps")
        nc.vector.memset(eps_t, 1e-5)
        # prewarm every activation-table set used, before anything queues
        NQ = 2                     # gather chunks
        QW = SOWN // NQ            # columns per chunk
        kv_in = [dram.tile([P, QW], BF16, name=f"kv_in{i}") for i in range(NQ)]
        kv_out = [dram.tile([2, P, QW], BF16, name=f"kv_out{i}")
                  for i in range(NQ)]
        warm = persist.tile([1, 4], F32, tag="warm")
        nc.scalar.activation(out=warm[:, 1:2], in_=eps_t[0:1, :], func=AF.Sqrt,
                             bias=eps_t[0:1, :])
        nc.scalar.activation(out=warm[:, 2:3], in_=eps_t[0:1, :],
                             func=AF.Identity, bias=eps_t[0:1, :])
        nc.scalar.activation(out=warm[:, 3:4], in_=eps_t[0:1, :], func=AF.Copy)
        nc.scalar.activation(out=warm[:, 0:1], in_=eps_t[0:1, :], func=AF.Exp)

        # ---------------- weights (host-packed; gpsimd DGE DMAs) --------
        wqkvT = persist.tile([P, KT, 3 * DA], BF16, tag="wqkvT")
        nc.gpsimd.dma_start(out=wqkvT, in_=wqkvT_d)
        qkvb = persist.tile([P, 3], F32, tag="qkvb")
        nc.gpsimd.dma_start(out=qkvb, in_=qkvb_d)
        csum = persist.tile([1, 256], BF16, tag="csum")
        nc.gpsimd.dma_start(out=csum, in_=csum_d)
        ldg_sb = persist.tile([P, KT, R], BF16, tag="ldg")
        nc.gpsimd.dma_start(out=ldg_sb, in_=ldg_d)
        LW2_sb = persist.tile([DA, R], BF16, tag="LW2")
        nc.gpsimd.dma_start(out=LW2_sb, in_=LW2_d)
        auglu_sb = persist.tile([AUG, DM], BF16, tag="auglu")
        nc.gpsimd.dma_start(out=auglu_sb, in_=auglu_d)
        WbT = persist.tile([P, KT, DM], BF16, tag="WbT")
        for k in range(KT):
            nc.gpsimd.dma_start(out=WbT[:, k, :], in_=WbT_d[:, k, :])

        # ---------------- persistent activations ----------------
        zhT = persist.tile([P, KT, SOWN], BF16, tag="zhT")    # z' = x*rstd, T
        qT = persist.tile([DA, SOWN], BF16, tag="qT")
        kv_own = persist.tile([P, SOWN], BF16, tag="kv_own")  # k 0:64, v 64:128
        kvT = persist.tile([P, SFULL], BF16, tag="kvT")       # global order
        kTt = kvT[0:DA, :]
        vT = kvT[DA:P, :]
        v_aug = [persist.tile([P, ST, 48], BF16, tag=f"vaug{h}", name=f"vaug{h}")
                 for h in range(NH)]
        aoT = persist.tile([DA, SOWN], BF16, tag="aoT")
        aug_tT = persist.tile([AUG, SOWN], BF16, tag="aug_tT")
        baseS = persist.tile([P, MT, DM], BF16, tag="baseS")
        stdAll = persist.tile([P, MT], F32, tag="stdAll")
        mrRow = persist.tile([1, SOWN], BF16, tag="mrRow")    # +mu*rstd
        rrS = persist.tile([HD, 512], F32, tag="rrS")
        nc.vector.memset(rrS, 0.0)

        for h in range(NH):
            nc.gpsimd.memset(v_aug[h][:, :, HD:HD + 1], 1.0)
        nc.gpsimd.memset(aug_tT, 0.0)
        nc.gpsimd.memset(aug_tT[32:33, :], 1.0)

        # ---------------- phase 1: x load + layernorm + z'^T ------------
        xin = persist.tile([P, MTO, E], BF16, tag="xin")
        for st in range(MTO):
            nc.sync.dma_start(out=xin[:, st, :],
                              in_=x_own[st * P:(st + 1) * P, :])

        def do_st(st):
            xf = xin[:, st, :]
            stats = st_pool.tile([P, 2, 6], F32, tag="bnstats")
            xr = xf.rearrange("p (n f) -> p n f", f=512)
            for sg in range(2):
                nc.vector.bn_stats(out=stats[:, sg, :], in_=xr[:, sg, :])
            mv = st_pool.tile([P, 2], F32, tag="mv")
            nc.vector.bn_aggr(out=mv, in_=stats)
            nc.scalar.activation(out=stdAll[:, st:st + 1], in_=mv[:, 1:2],
                                 func=AF.Sqrt, bias=eps_t)
            rstd = st_pool.tile([P, 1], F32, tag="rstd")
            nc.vector.reciprocal(out=rstd, in_=stdAll[:, st:st + 1])
            mr = st_pool.tile([P, 1], BF16, tag="mr")
            nc.scalar.activation(out=mr, in_=mv[:, 0:1], func=AF.Identity,
                                 scale=rstd)
            diag = dg_pool.tile([P, P], BF16, tag="diag")
            nc.vector.tensor_scalar(out=diag, in0=ident, scalar1=rstd,
                                    scalar2=None, op0=ALU.mult)
            smt = ps_sm()
            nc.tensor.matmul(smt[0:1, 0:P], mr, ident, start=True, stop=True)
            if st % 2 == 0:
                nc.vector.tensor_copy(out=mrRow[0:1, st * P:(st + 1) * P],
                                      in_=smt[0:1, 0:P])
            else:
                nc.scalar.copy(out=mrRow[0:1, st * P:(st + 1) * P],
                               in_=smt[0:1, 0:P])
            tpz = ps_tpz()
            tpf = tpz.rearrange("p j f -> p (j f)")
            for k in range(KT):
                nc.tensor.matmul(tpf[:, k * P:(k + 1) * P],
                                 xf[:, k * P:(k + 1) * P], diag,
                                 start=True, stop=True)
            tpk = tpz.rearrange("p j a -> p (j a)").rearrange(
                "p (k a) -> p k a", a=P)
            if st % 2 == 0:
                nc.vector.tensor_copy(out=zhT[:, :, st * P:(st + 1) * P],
                                      in_=tpk)
            else:
                nc.scalar.copy(out=zhT[:, :, st * P:(st + 1) * P], in_=tpk)

        # ---------------- qkv (own half only) ----------------
        def qk_own(grp):
            pq = ps_acc()
            for k in range(KT):
                nc.tensor.matmul(pq, wqkvT[:, k, 0:P],
                                 zhT[:, k, grp * 512:(grp + 1) * 512],
                                 start=(k == 0), stop=False)
            nc.tensor.matmul(pq, csum[0:1, 0:P],
                             mrRow[0:1, grp * 512:(grp + 1) * 512],
                             start=False, stop=True)
            nc.vector.tensor_scalar(
                out=qT[:, grp * 512:(grp + 1) * 512], in0=pq[0:DA, :],
                scalar1=qkvb[0:DA, 0:1], scalar2=None, op0=ALU.add)
            nc.scalar.activation(
                out=kv_own[0:DA, grp * 512:(grp + 1) * 512], in_=pq[DA:P, :],
                func=AF.Identity, bias=qkvb[DA:P, 0:1])

        def qkv_v(grp):
            pv = ps_acc()
            for k in range(KT):
                nc.tensor.matmul(pv[0:DA, :], wqkvT[:, k, P:3 * DA],
                                 zhT[:, k, grp * 512:(grp + 1) * 512],
                                 start=(k == 0), stop=False)
            nc.tensor.matmul(pv[0:DA, :], csum[0:1, P:3 * DA],
                             mrRow[0:1, grp * 512:(grp + 1) * 512],
                             start=False, stop=True)
            nc.vector.tensor_scalar(
                out=kv_own[DA:P, grp * 512:(grp + 1) * 512], in0=pv[0:DA, :],
                scalar1=qkvb[0:DA, 1:2], scalar2=None, op0=ALU.add)

        def vaug_grp(g):  # transpose v tiles jt = 4g..4g+3 (local order)
            tpv = ps_tpv()
            for j in range(4):
                jt = g * 4 + j
                nc.tensor.transpose(tpv[:, j * DA:(j + 1) * DA],
                                    vT[:, jt * P:(jt + 1) * P],
                                    ident[DA:P, DA:P])
            tv = tpv.rearrange("p (j d) -> p j d", d=DA)
            for h in range(NH):
                nc.vector.tensor_copy(
                    out=v_aug[h][:, g * 4:(g + 1) * 4, 0:HD],
                    in_=tv[:, 0:4, h * HD:(h + 1) * HD])

        # ---------------- base matmul tile ----------------
        in_attn = [False]
        pot_ps = [None]

        def pot_half(mt, g, half):
            if half == 0:
                pot_ps[0] = ps_accb() if in_attn[0] else ps_acc()
            pot = pot_ps[0]
            for k in range(half * 4, half * 4 + 4):
                nc.tensor.matmul(pot, zhT[:, k, mt * P:(mt + 1) * P],
                                 WbT[:, k, g * 512:(g + 1) * 512],
                                 start=(k == 0), stop=(k == KT - 1))
            if half == 1:
                if g == 0:
                    nc.vector.tensor_scalar(
                        out=baseS[:, mt, g * 512:(g + 1) * 512], in0=pot,
                        scalar1=stdAll[:, mt:mt + 1], scalar2=None,
                        op0=ALU.mult)
                else:
                    nc.scalar.activation(
                        out=baseS[:, mt, g * 512:(g + 1) * 512], in_=pot,
                        func=AF.Copy, scale=stdAll[:, mt:mt + 1])

        def pot_mt(mt, g):
            pot_half(mt, g, 0)
            pot_half(mt, g, 1)

        # ---------------- k/v pair exchange (quartered AllGather) -------
        # only the first collective pays the ~11us dispatch latency; the
        # rest queue behind it and pipeline on the pair link
        def kv_exchange(qc):
            c0 = qc * QW
            nc.sync.dma_start(out=kv_in[qc], in_=kv_own[:, c0:c0 + QW])
            nc.gpsimd.collective_compute(
                "AllGather", ALU.bypass,
                replica_groups=[[2 * i, 2 * i + 1] for i in range(NC // 2)],
                ins=[kv_in[qc].opt()], outs=[kv_out[qc].opt()])

        def kv_load(qc):
            kvv = kvT.rearrange("p (q g s) -> p q g s", q=NQ, g=2)
            nc.sync.dma_start(out=kvv[:, qc, :, :],
                              in_=kv_out[qc].rearrange("g p s -> p g s"))

        # ---------------- phase 1 schedule ----------------
        ps_ph1[0] = tc.alloc_tile_pool(name="ps_ph1", bufs=1, space="PSUM")
        extra = {3: [lambda: qk_own(0), lambda: qkv_v(0),
                     lambda: kv_exchange(0)],
                 4: [lambda: pot_mt(0, 0), lambda: pot_mt(0, 1)],
                 6: [lambda: pot_mt(1, 0), lambda: pot_mt(1, 1)],
                 7: [lambda: qk_own(1), lambda: qkv_v(1),
                     lambda: kv_exchange(1)]}
        for st in range(MTO):
            do_st(st)
            for fn in extra.get(st, []):
                fn()

        # cover the gather with two more base tiles, then load + transpose
        pot_half(2, 0, 0)
        pot_half(2, 0, 1)
        kv_load(0)
        pot_half(2, 1, 0)
        pot_half(2, 1, 1)
        pot_half(3, 0, 0)
        pot_half(3, 0, 1)
        kv_load(1)
        pot_half(3, 1, 0)
        pot_half(3, 1, 1)
        ps_ph1[0].release()

        in_attn[0] = True
        ps_attn[0] = tc.alloc_tile_pool(name="ps_attn", bufs=1, space="PSUM")
        vaug_grp(0)
        vaug_grp(1)

        # ---------------- attention (+ interleaved base / rest) ---------
        # skt pairs ordered so the first gather chunk's keys (global cols
        # 0:512 and 1024:1536) are consumed first; softmax is order-invariant
        # kvT is chunk-major: local tiles 0..7 are gather chunk 0, 8..15
        # chunk 1 (global positions scrambled; softmax is order-invariant)
        SKT_ORDER = list(range(ST))

        def attn_block(h, qg, fillers, preburst=()):
            d0 = h * HD
            pao = ps_pao()
            for fn in preburst:
                fn()
            exts = []
            nf = len(fillers)
            fi = 0

            def av(r):
                for j in range(2):
                    i = r * 2 + j
                    skt = SKT_ORDER[i]
                    nc.tensor.matmul(pao, v_aug[h][:, skt, 0:HD + 1],
                                     exts[r][:, j, :],
                                     start=(i == 0), stop=(i == ST - 1))

            for r in range(8):
                psc = ps_psc()
                for j in range(2):
                    skt = SKT_ORDER[r * 2 + j]
                    nc.tensor.matmul(psc[:, j, :],
                                     kTt[d0:d0 + HD, skt * P:(skt + 1) * P],
                                     qT[d0:d0 + HD, qg * 512:(qg + 1) * 512],
                                     start=True, stop=True)
                ext = ex_pool.tile([P, 2, 512], BF16, tag="ext")
                nc.scalar.activation(out=ext, in_=psc, func=AF.Exp,
                                     scale=ATT_SCALE)
                exts.append(ext)
                if r > 0:
                    av(r - 1)
                while fi * 8 < nf * (r + 1):
                    fillers[fi]()
                    fi += 1
            av(7)
            nc.vector.reciprocal(out=rrS[0:1, :], in_=pao[HD:HD + 1, :])
            rrb = st_pool.tile([HD, 512], F32, tag="rrb")
            # broadcast lane 0 across the 32-partition group on the DVE:
            # keeps the finalize on one engine (no Pool hop + sem trips)
            nc.vector.stream_shuffle(out=rrb, in_=rrS, mask=[0] * 32)
            nc.vector.tensor_tensor(
                out=aoT[d0:d0 + HD, qg * 512:(qg + 1) * 512],
                in0=pao[0:HD, :], in1=rrb, op=ALU.mult)
            while fi < nf:
                fillers[fi]()
                fi += 1

        tT_ps = [None]

        def tT_ldg(qg):
            p5 = ps_sm()[0:R, :]
            tT_ps[0] = p5
            for k in range(KT):
                nc.tensor.matmul(p5, ldg_sb[:, k, :],
                                 zhT[:, k, qg * 512:(qg + 1) * 512],
                                 start=(k == 0), stop=False)
            nc.tensor.matmul(p5, csum[0:1, 192:200],
                             mrRow[0:1, qg * 512:(qg + 1) * 512],
                             start=False, stop=False)

        def tT_h0(qg):
            nc.tensor.matmul(tT_ps[0], LW2_sb[0:HD, :],
                             aoT[0:HD, qg * 512:(qg + 1) * 512],
                             start=False, stop=False)

        def tT_part2(qg):
            p5 = tT_ps[0]
            nc.tensor.matmul(p5, LW2_sb[HD:DA, :],
                             aoT[HD:DA, qg * 512:(qg + 1) * 512],
                             start=False, stop=True)
            nc.vector.tensor_copy(
                out=aug_tT[0:R, qg * 512:(qg + 1) * 512], in_=p5)

        def rest_g(mt, g):
            racc = ps_accb()
            nc.tensor.matmul(racc, aug_tT[:, mt * P:(mt + 1) * P],
                             auglu_sb[:, g * 512:(g + 1) * 512],
                             start=True, stop=True)
            o_t = o_pool.tile([P, 512], BF16, tag="o_t")
            nc.vector.tensor_tensor(out=o_t, in0=racc,
                                    in1=baseS[:, mt, g * 512:(g + 1) * 512],
                                    op=ALU.add)
            nc.sync.dma_start(
                out=out_d[mt * P:(mt + 1) * P, g * 512:(g + 1) * 512],
                in_=o_t)

        def rest_tail_all(npre=0):
            # software-pipelined tail over (mt, g) items: ACT pre-copy of
            # baseS into psum, aug matmul accumulates on top, DVE copy out.
            # raccs ride the freed psc ring, two halves per alloc.
            items = [(mt, g) for mt in range(4, MT) for g in range(2)]
            raccs = [None] * len(items)
            pts = []
            for i, (mt, g) in enumerate(items):
                if i % 2 == 0:
                    pts.append(ps_psc())
                raccs[i] = pts[-1][:, i % 2, :]

            def pre(i):
                mt, g = items[i]
                nc.scalar.activation(
                    out=raccs[i], in_=baseS[:, mt, g * 512:(g + 1) * 512],
                    func=AF.Copy)

            def mm(i):
                mt, g = items[i]
                nc.tensor.matmul(raccs[i], aug_tT[:, mt * P:(mt + 1) * P],
                                 auglu_sb[:, g * 512:(g + 1) * 512],
                                 start=False, stop=True, skip_group_check=True)

            def out(i):
                mt, g = items[i]
                o_t = o_pool.tile([P, 512], BF16, tag="o_t")
                nc.vector.tensor_copy(out=o_t, in_=raccs[i])
                nc.sync.dma_start(
                    out=out_d[mt * P:(mt + 1) * P, g * 512:(g + 1) * 512],
                    in_=o_t)

            n = len(items)
            for i in range(npre):
                pre(i)
            for i in range(n + 2):
                if npre <= i < n:
                    pre(i)
                if 1 <= i < n + 1:
                    mm(i - 1)
                if i >= 2:
                    out(i - 2)

        attn_block(0, 0, [lambda: vaug_grp(2), lambda: vaug_grp(3),
                          lambda: pot_half(4, 0, 0), lambda: pot_half(4, 0, 1),
                          lambda: pot_half(4, 1, 0), lambda: pot_half(4, 1, 1)])
        attn_block(1, 0, [lambda: pot_half(5, 0, 0), lambda: pot_half(5, 0, 1),
                          lambda: pot_half(5, 1, 0), lambda: pot_half(5, 1, 1),
                          lambda: pot_half(6, 0, 0), lambda: pot_half(6, 0, 1)])
        attn_block(0, 1, [lambda: pot_half(6, 1, 0), lambda: pot_half(6, 1, 1),
                          lambda: tT_ldg(0), lambda: tT_h0(0),
                          lambda: tT_part2(0), lambda: rest_g(0, 0),
                          lambda: rest_g(0, 1), lambda: rest_g(1, 0)])
        attn_block(1, 1, [lambda: rest_g(1, 1), lambda: tT_ldg(1),
                          lambda: tT_h0(1), lambda: pot_half(7, 0, 0),
                          lambda: pot_half(7, 0, 1), lambda: rest_g(2, 0),
                          lambda: pot_half(7, 1, 0), lambda: pot_half(7, 1, 1),
                          lambda: rest_g(2, 1), lambda: rest_g(3, 0),
                          lambda: rest_g(3, 1)])
        tail_fns = rest_tail_all
        # stage the first tail pre-copies while the last finalize drains
        tT_part2(1)
        rest_tail_all(npre=3)
        ps_attn[0].release()

        if dbg:
            for nm, sb in [("zhT", zhT), ("qT", qT), ("kvT", kvT),
                           ("vaug0", v_aug[0]), ("vaug1", v_aug[1]),
                           ("aoT", aoT), ("augT", aug_tT),
                           ("baseS", baseS), ("mrRow", mrRow)]:
                nc.sync.dma_start(out=dbg_d[nm], in_=sb)

    nc.compile()
    return nc


_NC_CACHE = None


def _get_nc():
    global _NC_CACHE
    if _NC_CACHE is None:
        _NC_CACHE = build_kernel()
    return _NC_CACHE


def pack_weights(w_base, ln_gamma, ln_beta, lora_down, lora_up, w_qkv,
                 w_attn_out):
    """Host-side packing of all weights into device SBUF layouts."""
    f8 = np.float64
    g = np.asarray(ln_gamma, f8)
    b = np.asarray(ln_beta, f8)
    Wq = np.asarray(w_qkv, f8)                       # [192, E]
    Wqg = Wq * g[None, :]                            # gamma-folded
    wqkvT = Wqg.T.reshape(KT, P, 3 * DA).transpose(1, 0, 2)
    bqkv = b @ Wq.T                                  # [192]
    qkvb = np.zeros((P, 3), np.float32)
    qkvb[0:DA, 0] = bqkv[0:DA]                       # bq (partitions 0:64)
    qkvb[DA:P, 0] = bqkv[DA:2 * DA]                  # bk (partitions 64:128)
    qkvb[0:DA, 1] = bqkv[2 * DA:3 * DA]              # bv (partitions 0:64)
    qkvb[0:DA, 2] = bqkv[DA:2 * DA]                  # bk (partitions 0:64)
    Wb = np.asarray(w_base, f8)                      # [DM, E]
    WbT = Wb.T.reshape(KT, P, DM).transpose(1, 0, 2)
    ld = np.asarray(lora_down, f8)                   # [E, R]
    ldgm = ld * g[:, None]                           # gamma-folded
    ldg = ldgm.reshape(KT, P, R).transpose(1, 0, 2)
    LW2 = np.asarray(w_attn_out, f8).T @ ld          # [DA, R]
    lu = np.asarray(lora_up, f8)                     # [R, DM]
    csum = np.zeros((1, 256), f8)
    # negated: device row is +mu*rstd, correction subtracts colsum * mr
    csum[0, 0:3 * DA] = -Wqg.sum(axis=1)
    csum[0, 3 * DA:3 * DA + R] = -ldgm.sum(axis=0)
    auglu = np.zeros((AUG, DM), f8)
    auglu[0:R] = SCALING * lu
    auglu[32] = SCALING * ((b @ ld) @ lu)            # beta const (pairs ones)

    def cast(a):
        return np.ascontiguousarray(a.astype(BF16NP))

    return {"wqkvT": cast(wqkvT), "qkvb": np.ascontiguousarray(qkvb),
            "csum": cast(csum), "WbT": cast(WbT), "ldg": cast(ldg),
            "LW2": cast(LW2), "auglu": cast(auglu)}


def core_in_maps(x, wk):
    xb = np.asarray(x).astype(BF16NP)
    in_maps = []
    for c in range(NC):
        b, half = divmod(c, 2)
        own = np.ascontiguousarray(xb[b, half * SOWN:(half + 1) * SOWN])
        in_maps.append({"x_own": own, **wk})
    return in_maps


def kernel(x, w_base, ln_gamma, ln_beta, lora_down, lora_up, w_qkv, w_attn_out,
           _trace=False):
    wk = pack_weights(w_base, ln_gamma, ln_beta, lora_down, lora_up, w_qkv,
                      w_attn_out)
    nc = _get_nc()
    in_maps = core_in_maps(x, wk)
    res = run_bass_kernel_spmd(nc, in_maps, core_ids=list(range(NC)),
                               trace=_trace)
    B, S = 4, SFULL
    out = np.empty((B, S, DM), np.float32)
    for c in range(NC):
        b, half = divmod(c, 2)
        out[b, half * SOWN:(half + 1) * SOWN] = np.asarray(
            res.results[c]["out"], dtype=np.float32)
    if _trace:
        kernel.last_exec_time_ns = res.exec_time_ns
        kernel.last_results = res
    return out
